# revision 2
# baseline (speedup 1.0000x reference)
"""Sigmoid-attention MHA kernel for 8 Trainium2 NeuronCores.

Problem: x[4,2048,512], W_q/W_k/W_v/W_o[512,512] (already scaled).
  Q = x@Wq.T, K = x@Wk.T, V = x@Wv.T split into 8 heads of depth 64
  attn = sigmoid(QK^T/sqrt(64) - log(2048));  out = (attn@V merged)@Wo.T

Sharding: core c handles batch b=c//2, head-group g=c%2 (4 heads each).
Each core computes a partial output projection over its 256 head-features;
host sums the two partials per batch.

All PE operands are bf16 (host-converted); PSUM accumulation is fp32.
Attention matmuls use PE array tiling for 2x concurrency:
  scores: two heads' K=64 matmuls at row positions (0,0)/(64,0)
  attn@V: two heads' M=64 matmuls at col positions (0,0)/(0,64)
   (tile_position auto-derives from lhsT/out base partitions)
Sigmoid runs on ScalarE from PSUM [128,1024] tiles -> bf16 SBUF; ScalarE
is the bottleneck engine (~1.1us per tile x 128 tiles per iteration).

KERNEL_LOOP>0 unrolls the body N times (python loop, no barriers) with
cross-iteration software pipelining: iteration k+1's input DMA and
Q/K/V projections are interleaved into iteration k's attention loop
(x/q/k/v tiles double-buffered), so a timing run measures steady-state
throughput.
"""

import os
import numpy as np
import ml_dtypes

LOOP = int(os.environ.get("KERNEL_LOOP", "0"))
SBUFS = int(os.environ.get("KERNEL_SB", "3"))   # scores psum bufs
ABUFS = int(os.environ.get("KERNEL_AB", "6"))   # attn sbuf bufs

B, S, D = 4, 2048, 512
NH, DEPTH = 8, 64
G = 2          # head groups (one per core pair)
GF = 256       # features per group
NEG_LOG_S = float(np.float32(-np.log(np.float32(S))))
INV_SQRT_DK = 0.125
BF16_NP = np.dtype(ml_dtypes.bfloat16)

_CACHE = {}


def _build_nc():
    import concourse.bacc as bacc
    import concourse.tile as tile
    from concourse import mybir

    f32 = mybir.dt.float32
    bf16 = mybir.dt.bfloat16
    nc = bacc.Bacc("TRN2", target_bir_lowering=False, debug=False, num_devices=8)

    xt_d = nc.dram_tensor("xt", [128, 8192], bf16, kind="ExternalInput").ap()
    wq_d = nc.dram_tensor("wq", [128, 1024], bf16, kind="ExternalInput").ap()
    wk_d = nc.dram_tensor("wk", [128, 1024], bf16, kind="ExternalInput").ap()
    wv_d = nc.dram_tensor("wv", [128, 1024], bf16, kind="ExternalInput").ap()
    wo_d = nc.dram_tensor("wo", [128, 1024], bf16, kind="ExternalInput").ap()
    out_d = nc.dram_tensor("out", [S, D], f32, kind="ExternalOutput").ap()

    iters = max(LOOP, 1)
    NB = 2 if iters > 1 else 1  # buffer sets for cross-iteration pipelining

    with tile.TileContext(nc) as tc:
        with (
            tc.tile_pool(name="persist", bufs=1) as persist,
            tc.tile_pool(name="attn", bufs=ABUFS) as apool,
            tc.tile_pool(name="stage", bufs=4) as stage,
            tc.tile_pool(name="spsum", bufs=SBUFS, space="PSUM") as spsum,
            tc.tile_pool(name="opsum", bufs=2, space="PSUM") as opsum,
        ):
            Sig = mybir.ActivationFunctionType.Sigmoid

            def mm(out, lhsT, rhs, start, stop):
                nc.tensor.matmul(out, lhsT=lhsT, rhs=rhs, start=start, stop=stop)

            bias_t = persist.tile([128, 1], f32, tag="bias", name="bias_t")
            nc.vector.memset(bias_t[:], NEG_LOG_S)
            warm_t = persist.tile([128, 1], f32, tag="warm", name="warm_t")
            nc.scalar.activation(warm_t[:], bias_t[:], Sig, bias=bias_t[:])

            wq_sb = persist.tile([128, 1024], bf16, tag="wq", name="wq_sb")
            wk_sb = persist.tile([128, 1024], bf16, tag="wk", name="wk_sb")
            wv_sb = persist.tile([128, 1024], bf16, tag="wv", name="wv_sb")
            wo_sb = persist.tile([128, 1024], bf16, tag="wo", name="wo_sb")

            class BufSet:
                pass

            def make_bufset(s):
                bs = BufSet()
                bs.xt = [persist.tile([128, 2048], bf16, tag=f"xt{s}_{c}",
                                      name=f"xt{s}_{c}") for c in range(4)]
                bs.qt = [persist.tile([128, 2048], bf16, tag=f"qt{s}_{m}",
                                      name=f"qt{s}_{m}") for m in range(2)]
                bs.kt = [persist.tile([128, 2048], bf16, tag=f"kt{s}_{m}",
                                      name=f"kt{s}_{m}") for m in range(2)]
                bs.v = [persist.tile([128, 256], bf16, tag=f"v{s}_{t}",
                                     name=f"v{s}_{t}") for t in range(16)]
                return bs

            bsets = [make_bufset(s) for s in range(NB)]
            # ot is intra-iteration only: written by attention, read by the
            # interleaved output-projection waves of the same iteration.
            ot = [persist.tile([128, 2048], bf16, tag=f"ot{m}", name=f"ot{m}")
                  for m in range(2)]

            def dma_in(bs):
                for c in range(4):
                    nc.sync.dma_start(out=bs.xt[c][:],
                                      in_=xt_d[:, 2048 * c:2048 * (c + 1)])
                nc.sync.dma_start(out=wq_sb[:], in_=wq_d[:])
                nc.sync.dma_start(out=wk_sb[:], in_=wk_d[:])
                nc.sync.dma_start(out=wv_sb[:], in_=wv_d[:])
                nc.sync.dma_start(out=wo_sb[:], in_=wo_d[:])

            def proj_closures(bs):
                """24 closures, each emitting one PSUM group of the Q/K/V
                projections for buffer set bs. Spread across the previous
                iteration's attention loop."""
                fns = []
                for mc in range(2):
                    for w_sb, dsts in ((wq_sb, bs.qt), (wk_sb, bs.kt)):
                        for qh in range(2):
                            def f(mc=mc, w_sb=w_sb, dst=dsts[mc], qh=qh):
                                ps = spsum.tile([128, 1024], f32, tag="s",
                                                name="pp")
                                for qsub in range(2):
                                    qc = 2 * qh + qsub
                                    col = slice(512 * qsub, 512 * (qsub + 1))
                                    for kc in range(4):
                                        w0 = 256 * kc + 128 * mc
                                        mm(ps[:, col],
                                           w_sb[:, w0:w0 + 128],
                                           bs.xt[kc][:, 512 * qc:512 * (qc + 1)],
                                           start=(kc == 0), stop=(kc == 3))
                                nc.vector.tensor_copy(
                                    dst[:, 1024 * qh:1024 * (qh + 1)], ps[:, :])
                            fns.append(f)
                for tck in range(16):
                    def f(tck=tck):
                        pv = spsum.tile([128, 1024], f32, tag="s", name="pv")
                        for vkc in range(4):
                            mm(pv[:, 0:256],
                               bs.xt[vkc][:, 128 * tck:128 * (tck + 1)],
                               wv_sb[:, 256 * vkc:256 * (vkc + 1)],
                               start=(vkc == 0), stop=(vkc == 3))
                        nc.vector.tensor_copy(bs.v[tck][:], pv[:, 0:256])
                    fns.append(f)
                return fns

            def p_wave(wave):
                # output projection for tokens [256*wave, 256*(wave+1))
                st = stage.tile([128, 2, 512], f32, tag="pstage", name="pstage")
                ps = spsum.tile([128, 1024], f32, tag="s", name="po")
                for half in range(2):
                    tck = 2 * wave + half
                    col = slice(512 * half, 512 * (half + 1))
                    for c in range(2):
                        mm(ps[:, col],
                           ot[c][:, 128 * tck:128 * (tck + 1)],
                           wo_sb[:, 512 * c:512 * (c + 1)],
                           start=(c == 0), stop=(c == 1))
                nc.vector.tensor_copy(st[:, :, :], ps[:, 0:1024])
                dst = out_d[256 * wave:256 * (wave + 1), :].rearrange(
                    "(t p) m -> p t m", p=128)
                nc.sync.dma_start(out=dst, in_=st[:])

            def attention(bs, inter):
                """Flat software-pipelined loop over (qc, p, kc); scores for
                i+1 are emitted before sigmoid(i)/attnV(i) so the PE keeps
                ScalarE fed. Output-projection waves run per finished qc.
                `inter` closures (next iteration's DMA + projections) are
                spread across the loop."""
                flat = [(qc, p, kc)
                        for qc in range(4) for p in range(2) for kc in range(16)]
                spread = {}
                if inter:
                    idxs = np.linspace(6, 120, len(inter)).astype(int)
                    for j, fn in enumerate(inter):
                        spread.setdefault(int(idxs[j]), []).append(fn)

                def emit_scores(qc, p, kc):
                    sp = spsum.tile([128, 1024], f32, tag="s", name="ps")
                    ks = slice(128 * kc, 128 * (kc + 1))
                    qs = slice(512 * qc, 512 * (qc + 1))
                    mm(sp[:, 0:512], bs.kt[p][0:64, ks], bs.qt[p][0:64, qs],
                       start=True, stop=True)
                    mm(sp[:, 512:1024], bs.kt[p][64:128, ks],
                       bs.qt[p][64:128, qs], start=True, stop=True)
                    return sp

                psO = None
                s_cur = emit_scores(*flat[0])
                for i, (qc, p, kc) in enumerate(flat):
                    s_nxt = emit_scores(*flat[i + 1]) if i + 1 < len(flat) else None
                    a = apool.tile([128, 1024], bf16, tag="a", name="attn")
                    nc.scalar.activation(a[:], s_cur[:], Sig,
                                         bias=bias_t[:], scale=INV_SQRT_DK)
                    if kc == 0:
                        psO = opsum.tile([128, 512], f32, tag="o", name="psO")
                    pb = 128 * p
                    mm(psO[0:64, :], bs.v[kc][:, pb:pb + 64],
                       a[:, 0:512], start=(kc == 0), stop=(kc == 15))
                    mm(psO[64:128, :], bs.v[kc][:, pb + 64:pb + 128],
                       a[:, 512:1024], start=(kc == 0), stop=(kc == 15))
                    if kc == 15:
                        qs = slice(512 * qc, 512 * (qc + 1))
                        nc.vector.tensor_copy(ot[p][:, qs], psO[:, :])
                        if p == 1:
                            p_wave(2 * qc)
                            p_wave(2 * qc + 1)
                    for fn in spread.get(i, []):
                        fn()
                    s_cur = s_nxt

            # prologue: iteration 0's inputs + projections
            dma_in(bsets[0])
            for fn in proj_closures(bsets[0]):
                fn()
            for k in range(iters):
                cur = bsets[k % NB]
                inter = []
                if k + 1 < iters:
                    nxt = bsets[(k + 1) % NB]
                    inter = [lambda bs=nxt: dma_in(bs)] + proj_closures(nxt)
                attention(cur, inter)

    nc.compile()
    return nc


def get_nc():
    if "nc" not in _CACHE:
        _CACHE["nc"] = _build_nc()
    return _CACHE["nc"]


def make_in_maps(x, W_q, W_k, W_v, W_o):
    x = np.ascontiguousarray(np.asarray(x, dtype=np.float32))
    ws = [np.asarray(w, dtype=np.float32) for w in (W_q, W_k, W_v, W_o)]
    W_q, W_k, W_v, W_o = ws

    def chunked(a, nchunks):
        # [128*nchunks, m] -> [128, nchunks*m] with chunk-major columns
        m = a.shape[1]
        return np.ascontiguousarray(
            a.reshape(nchunks, 128, m).transpose(1, 0, 2).reshape(128, nchunks * m)
        ).astype(BF16_NP)

    in_maps = []
    for c in range(8):
        b, g = divmod(c, 2)
        gf = slice(GF * g, GF * (g + 1))
        in_maps.append({
            "xt": chunked(np.ascontiguousarray(x[b].T), 4),
            "wq": chunked(np.ascontiguousarray(W_q[gf, :].T), 4),
            "wk": chunked(np.ascontiguousarray(W_k[gf, :].T), 4),
            "wv": chunked(np.ascontiguousarray(W_v[gf, :].T), 4),
            "wo": chunked(np.ascontiguousarray(W_o[:, gf].T), 2),
        })
    return in_maps


def kernel(x, W_q, W_k, W_v, W_o):
    from concourse.bass_utils import run_bass_kernel_spmd

    nc = get_nc()
    in_maps = make_in_maps(x, W_q, W_k, W_v, W_o)
    res = run_bass_kernel_spmd(nc, in_maps, list(range(8)))
    parts = [res.results[c]["out"] for c in range(8)]
    out = np.stack([parts[2 * b] + parts[2 * b + 1] for b in range(B)])
    return np.ascontiguousarray(out.astype(np.float32))


# revision 5
# speedup vs baseline: 30.7472x; 30.7472x over previous
"""Sigmoid-attention MHA kernel for 8 Trainium2 NeuronCores.

Problem: x[4,2048,512], W_q/W_k/W_v/W_o[512,512] (already scaled).
  Q = x@Wq.T, K = x@Wk.T, V = x@Wv.T split into 8 heads of depth 64
  attn = sigmoid(QK^T/sqrt(64) - log(2048));  out = (attn@V merged)@Wo.T

Sharding: core c handles batch b=c//2, head-group g=c%2 (4 heads each).
Each core computes a partial output projection over its 256 head-features;
host sums the two partials per batch.

All PE operands are bf16 (host-converted); PSUM accumulation is fp32.
Attention matmuls use PE array tiling for 2x concurrency:
  scores: two heads' K=64 matmuls at row positions (0,0)/(64,0)
  attn@V: two heads' M=64 matmuls at col positions (0,0)/(0,64)
   (tile_position auto-derives from lhsT/out base partitions)
Sigmoid runs on ScalarE from PSUM [128,1024] tiles -> bf16 SBUF; ScalarE
is the bottleneck engine (~1.1us per tile x 128 tiles per iteration).

KERNEL_LOOP>0 unrolls the body N times (python loop, no barriers) with
cross-iteration software pipelining: iteration k+1's input DMA and
Q/K/V projections are interleaved into iteration k's attention loop
(x/q/k/v tiles double-buffered), so a timing run measures steady-state
throughput.
"""

import os
import numpy as np
import ml_dtypes

LOOP = int(os.environ.get("KERNEL_LOOP", "0"))  # For_i trip count (timing)
UNROLL = int(os.environ.get("KERNEL_UNROLL", "4"))  # bodies per For_i trip
SBUFS = int(os.environ.get("KERNEL_SB", "3"))   # scores psum bufs
ABUFS = int(os.environ.get("KERNEL_AB", "6"))   # attn sbuf bufs

B, S, D = 4, 2048, 512
NH, DEPTH = 8, 64
G = 2          # head groups (one per core pair)
GF = 256       # features per group
NEG_LOG_S = float(np.float32(-np.log(np.float32(S))))
INV_SQRT_DK = 0.125
BF16_NP = np.dtype(ml_dtypes.bfloat16)

_CACHE = {}


def _build_nc():
    import concourse.bacc as bacc
    import concourse.tile as tile
    from concourse import mybir

    f32 = mybir.dt.float32
    bf16 = mybir.dt.bfloat16
    nc = bacc.Bacc("TRN2", target_bir_lowering=False, debug=False, num_devices=8)

    xt_d = nc.dram_tensor("xt", [128, 8192], bf16, kind="ExternalInput").ap()
    wq_d = nc.dram_tensor("wq", [128, 1024], bf16, kind="ExternalInput").ap()
    wk_d = nc.dram_tensor("wk", [128, 1024], bf16, kind="ExternalInput").ap()
    wv_d = nc.dram_tensor("wv", [128, 1024], bf16, kind="ExternalInput").ap()
    wo_d = nc.dram_tensor("wo", [128, 1024], bf16, kind="ExternalInput").ap()
    out_d = nc.dram_tensor("out", [S, D], f32, kind="ExternalOutput").ap()

    # LOOP>0: For_i(0, LOOP) around UNROLL pipelined bodies, so a timing run
    # measures steady-state throughput with the loop barrier amortized 1/UNROLL.
    iters = UNROLL if LOOP > 0 else 1
    NB = 2 if iters > 1 else 1  # buffer sets for cross-iteration pipelining

    with tile.TileContext(nc) as tc:
        with (
            tc.tile_pool(name="persist", bufs=1) as persist,
            tc.tile_pool(name="attn", bufs=ABUFS) as apool,
            tc.tile_pool(name="stage", bufs=4) as stage,
            tc.tile_pool(name="spsum", bufs=SBUFS, space="PSUM") as spsum,
            tc.tile_pool(name="opsum", bufs=2, space="PSUM") as opsum,
        ):
            Sig = mybir.ActivationFunctionType.Sigmoid

            def mm(out, lhsT, rhs, start, stop):
                nc.tensor.matmul(out, lhsT=lhsT, rhs=rhs, start=start, stop=stop)

            bias_t = persist.tile([128, 1], f32, tag="bias", name="bias_t")
            nc.vector.memset(bias_t[:], NEG_LOG_S)
            warm_t = persist.tile([128, 1], f32, tag="warm", name="warm_t")
            nc.scalar.activation(warm_t[:], bias_t[:], Sig, bias=bias_t[:])

            wq_sb = persist.tile([128, 1024], bf16, tag="wq", name="wq_sb")
            wk_sb = persist.tile([128, 1024], bf16, tag="wk", name="wk_sb")
            wv_sb = persist.tile([128, 1024], bf16, tag="wv", name="wv_sb")
            wo_sb = persist.tile([128, 1024], bf16, tag="wo", name="wo_sb")

            class BufSet:
                pass

            def make_bufset(s):
                bs = BufSet()
                bs.xt = [persist.tile([128, 2048], bf16, tag=f"xt{s}_{c}",
                                      name=f"xt{s}_{c}") for c in range(4)]
                bs.qt = [persist.tile([128, 2048], bf16, tag=f"qt{s}_{m}",
                                      name=f"qt{s}_{m}") for m in range(2)]
                bs.kt = [persist.tile([128, 2048], bf16, tag=f"kt{s}_{m}",
                                      name=f"kt{s}_{m}") for m in range(2)]
                bs.v = [persist.tile([128, 256], bf16, tag=f"v{s}_{t}",
                                     name=f"v{s}_{t}") for t in range(16)]
                return bs

            bsets = [make_bufset(s) for s in range(NB)]
            # ot is intra-iteration only: written by attention, read by the
            # interleaved output-projection waves of the same iteration.
            ot = [persist.tile([128, 2048], bf16, tag=f"ot{m}", name=f"ot{m}")
                  for m in range(2)]

            def dma_in(bs):
                for c in range(4):
                    nc.sync.dma_start(out=bs.xt[c][:],
                                      in_=xt_d[:, 2048 * c:2048 * (c + 1)])
                nc.sync.dma_start(out=wq_sb[:], in_=wq_d[:])
                nc.sync.dma_start(out=wk_sb[:], in_=wk_d[:])
                nc.sync.dma_start(out=wv_sb[:], in_=wv_d[:])
                nc.sync.dma_start(out=wo_sb[:], in_=wo_d[:])

            def proj_closures(bs):
                """24 closures, each emitting one PSUM group of the Q/K/V
                projections for buffer set bs. Spread across the previous
                iteration's attention loop."""
                fns = []
                for mc in range(2):
                    for w_sb, dsts in ((wq_sb, bs.qt), (wk_sb, bs.kt)):
                        for qh in range(2):
                            def f(mc=mc, w_sb=w_sb, dst=dsts[mc], qh=qh):
                                ps = spsum.tile([128, 1024], f32, tag="s",
                                                name="pp")
                                for qsub in range(2):
                                    qc = 2 * qh + qsub
                                    col = slice(512 * qsub, 512 * (qsub + 1))
                                    for kc in range(4):
                                        w0 = 256 * kc + 128 * mc
                                        mm(ps[:, col],
                                           w_sb[:, w0:w0 + 128],
                                           bs.xt[kc][:, 512 * qc:512 * (qc + 1)],
                                           start=(kc == 0), stop=(kc == 3))
                                nc.vector.tensor_copy(
                                    dst[:, 1024 * qh:1024 * (qh + 1)], ps[:, :])
                            fns.append(f)
                for tck in range(16):
                    def f(tck=tck):
                        pv = spsum.tile([128, 1024], f32, tag="s", name="pv")
                        for vkc in range(4):
                            mm(pv[:, 0:256],
                               bs.xt[vkc][:, 128 * tck:128 * (tck + 1)],
                               wv_sb[:, 256 * vkc:256 * (vkc + 1)],
                               start=(vkc == 0), stop=(vkc == 3))
                        nc.vector.tensor_copy(bs.v[tck][:], pv[:, 0:256])
                    fns.append(f)
                return fns

            def p_wave(wave):
                # output projection for tokens [256*wave, 256*(wave+1))
                st = stage.tile([128, 2, 512], f32, tag="pstage", name="pstage")
                ps = spsum.tile([128, 1024], f32, tag="s", name="po")
                for half in range(2):
                    tck = 2 * wave + half
                    col = slice(512 * half, 512 * (half + 1))
                    for c in range(2):
                        mm(ps[:, col],
                           ot[c][:, 128 * tck:128 * (tck + 1)],
                           wo_sb[:, 512 * c:512 * (c + 1)],
                           start=(c == 0), stop=(c == 1))
                nc.vector.tensor_copy(st[:, :, :], ps[:, 0:1024])
                dst = out_d[256 * wave:256 * (wave + 1), :].rearrange(
                    "(t p) m -> p t m", p=128)
                nc.sync.dma_start(out=dst, in_=st[:])

            def attention(bs, inter):
                """Flat software-pipelined loop over (qc, p, kc); scores for
                i+1 are emitted before sigmoid(i)/attnV(i) so the PE keeps
                ScalarE fed. Output-projection waves run per finished qc.
                `inter` closures (next iteration's DMA + projections) are
                spread across the loop."""
                flat = [(qc, p, kc)
                        for qc in range(4) for p in range(2) for kc in range(16)]
                spread = {}
                if inter:
                    idxs = np.linspace(6, 120, len(inter)).astype(int)
                    for j, fn in enumerate(inter):
                        spread.setdefault(int(idxs[j]), []).append(fn)

                def emit_scores(qc, p, kc):
                    sp = spsum.tile([128, 1024], f32, tag="s", name="ps")
                    ks = slice(128 * kc, 128 * (kc + 1))
                    qs = slice(512 * qc, 512 * (qc + 1))
                    mm(sp[:, 0:512], bs.kt[p][0:64, ks], bs.qt[p][0:64, qs],
                       start=True, stop=True)
                    mm(sp[:, 512:1024], bs.kt[p][64:128, ks],
                       bs.qt[p][64:128, qs], start=True, stop=True)
                    return sp

                psO = None
                s_cur = emit_scores(*flat[0])
                for i, (qc, p, kc) in enumerate(flat):
                    s_nxt = emit_scores(*flat[i + 1]) if i + 1 < len(flat) else None
                    a = apool.tile([128, 1024], bf16, tag="a", name="attn")
                    nc.scalar.activation(a[:], s_cur[:], Sig,
                                         bias=bias_t[:], scale=INV_SQRT_DK)
                    if kc == 0:
                        psO = opsum.tile([128, 512], f32, tag="o", name="psO")
                    pb = 128 * p
                    mm(psO[0:64, :], bs.v[kc][:, pb:pb + 64],
                       a[:, 0:512], start=(kc == 0), stop=(kc == 15))
                    mm(psO[64:128, :], bs.v[kc][:, pb + 64:pb + 128],
                       a[:, 512:1024], start=(kc == 0), stop=(kc == 15))
                    if kc == 15:
                        qs = slice(512 * qc, 512 * (qc + 1))
                        nc.vector.tensor_copy(ot[p][:, qs], psO[:, :])
                        if p == 1:
                            p_wave(2 * qc)
                            p_wave(2 * qc + 1)
                    for fn in spread.get(i, []):
                        fn()
                    s_cur = s_nxt

            def body():
                # prologue: iteration 0's inputs + projections
                dma_in(bsets[0])
                for fn in proj_closures(bsets[0]):
                    fn()
                for k in range(iters):
                    cur = bsets[k % NB]
                    inter = []
                    if k + 1 < iters:
                        nxt = bsets[(k + 1) % NB]
                        inter = [lambda bs=nxt: dma_in(bs)] + proj_closures(nxt)
                    attention(cur, inter)

            if LOOP > 0:
                hint = tuple(mybir.EngineType[e] for e in
                             ("PE", "Activation", "DVE", "SP", "Pool"))
                with tc.For_i(0, LOOP, 1, hint_engines=hint):
                    body()
            else:
                body()

    nc.compile()
    return nc


def get_nc():
    if "nc" not in _CACHE:
        _CACHE["nc"] = _build_nc()
    return _CACHE["nc"]


def make_in_maps(x, W_q, W_k, W_v, W_o):
    x = np.ascontiguousarray(np.asarray(x, dtype=np.float32))
    ws = [np.asarray(w, dtype=np.float32) for w in (W_q, W_k, W_v, W_o)]
    W_q, W_k, W_v, W_o = ws

    def chunked(a, nchunks):
        # [128*nchunks, m] -> [128, nchunks*m] with chunk-major columns
        m = a.shape[1]
        return np.ascontiguousarray(
            a.reshape(nchunks, 128, m).transpose(1, 0, 2).reshape(128, nchunks * m)
        ).astype(BF16_NP)

    in_maps = []
    for c in range(8):
        b, g = divmod(c, 2)
        gf = slice(GF * g, GF * (g + 1))
        in_maps.append({
            "xt": chunked(np.ascontiguousarray(x[b].T), 4),
            "wq": chunked(np.ascontiguousarray(W_q[gf, :].T), 4),
            "wk": chunked(np.ascontiguousarray(W_k[gf, :].T), 4),
            "wv": chunked(np.ascontiguousarray(W_v[gf, :].T), 4),
            "wo": chunked(np.ascontiguousarray(W_o[:, gf].T), 2),
        })
    return in_maps


def kernel(x, W_q, W_k, W_v, W_o):
    from concourse.bass_utils import run_bass_kernel_spmd

    nc = get_nc()
    in_maps = make_in_maps(x, W_q, W_k, W_v, W_o)
    res = run_bass_kernel_spmd(nc, in_maps, list(range(8)))
    parts = [res.results[c]["out"] for c in range(8)]
    out = np.stack([parts[2 * b] + parts[2 * b + 1] for b in range(B)])
    return np.ascontiguousarray(out.astype(np.float32))


# revision 13
# speedup vs baseline: 37.2169x; 1.2104x over previous
"""Sigmoid-attention MHA kernel for 8 Trainium2 NeuronCores.

Problem: x[4,2048,512], W_q/W_k/W_v/W_o[512,512] (already scaled).
  Q = x@Wq.T, K = x@Wk.T, V = x@Wv.T split into 8 heads of depth 64
  attn = sigmoid(QK^T/sqrt(64) - log(2048));  out = (attn@V merged)@Wo.T

Sharding: core c handles batch b=c//2, head-group g=c%2 (4 heads each).
Each core computes a partial output projection over its 256 head-features;
host sums the two partials per batch.

All PE operands are bf16 (host-converted); PSUM accumulation is fp32.
Attention matmuls use PE array tiling for 2x concurrency:
  scores: two heads' K=64 matmuls at row positions (0,0)/(64,0)
  attn@V: two heads' M=64 matmuls at col positions (0,0)/(0,64)
   (tile_position auto-derives from lhsT/out base partitions)
Sigmoid runs on ScalarE from PSUM [128,1024] tiles -> bf16 SBUF; ScalarE
is the bottleneck engine (~1.1us per tile x 128 tiles per iteration).

KERNEL_LOOP>0 unrolls the body N times (python loop, no barriers) with
cross-iteration software pipelining: iteration k+1's input DMA and
Q/K/V projections are interleaved into iteration k's attention loop
(x/q/k/v tiles double-buffered), so a timing run measures steady-state
throughput.
"""

import os
import numpy as np
import ml_dtypes

LOOP = int(os.environ.get("KERNEL_LOOP", "0"))  # For_i trip count (timing)
UNROLL = int(os.environ.get("KERNEL_UNROLL", "8"))  # bodies per For_i trip
ABUFS = int(os.environ.get("KERNEL_AB", "6"))   # attn sbuf bufs
# sigmoid tile width (512-col score matmul outputs per ScalarE activation);
# 1536 amortizes the ~350-cycle per-instruction ACT overhead over 3 outputs.
ACTW = int(os.environ.get("KERNEL_ACTW", "1024"))
W = ACTW // 512

B, S, D = 4, 2048, 512
NH, DEPTH = 8, 64
G = 2          # head groups (one per core pair)
GF = 256       # features per group
NEG_LOG_S = float(np.float32(-np.log(np.float32(S))))
INV_SQRT_DK = 0.125
BF16_NP = np.dtype(ml_dtypes.bfloat16)

_CACHE = {}


def _build_nc():
    import concourse.bacc as bacc
    import concourse.tile as tile
    from concourse import mybir

    f32 = mybir.dt.float32
    bf16 = mybir.dt.bfloat16
    nc = bacc.Bacc("TRN2", target_bir_lowering=False, debug=False, num_devices=8)

    xt_d = nc.dram_tensor("xt", [128, 8192], bf16, kind="ExternalInput").ap()
    wq_d = nc.dram_tensor("wq", [128, 1024], bf16, kind="ExternalInput").ap()
    wk_d = nc.dram_tensor("wk", [128, 1024], bf16, kind="ExternalInput").ap()
    wv_d = nc.dram_tensor("wv", [128, 1024], bf16, kind="ExternalInput").ap()
    wo_d = nc.dram_tensor("wo", [128, 1024], bf16, kind="ExternalInput").ap()
    out_d = nc.dram_tensor("out", [S, D], f32, kind="ExternalOutput").ap()

    # LOOP>0: For_i(0, LOOP) around UNROLL pipelined bodies, so a timing run
    # measures steady-state throughput with the loop barrier amortized 1/UNROLL.
    # KERNEL_FORI=0: python-unroll LOOP bodies instead (TimelineSim can't
    # follow For_i register branches).
    fori = bool(int(os.environ.get("KERNEL_FORI", "1"))) and LOOP > 0
    iters = UNROLL if fori else max(LOOP, 1)
    NB = 2 if iters > 1 else 1  # buffer sets for cross-iteration pipelining

    # PSUM budget (8 banks of 512 fp32):
    #   W=3: scores [128,1536] x2 (6 banks) + psO [128,512] x1 + proj x1
    #   W=2: scores [128,1024] x3 (6 banks) + psO [128,512] x2, proj shares "s"
    sb = {2: 3, 3: 2}[W]
    ob = {2: 2, 3: 1}[W]
    ptag = "s" if W == 2 else "pj"
    pbufs = None if W == 2 else 1  # the pj tag gets a single PSUM bank

    with tile.TileContext(nc) as tc:
        with (
            tc.tile_pool(name="persist", bufs=1) as persist,
            tc.tile_pool(name="attn", bufs=ABUFS) as apool,
            tc.tile_pool(name="stage", bufs=4) as stage,
            tc.tile_pool(name="spsum", bufs=sb, space="PSUM") as spsum,
            tc.tile_pool(name="opsum", bufs=ob, space="PSUM") as opsum,
        ):
            Sig = mybir.ActivationFunctionType.Sigmoid

            def mm(out, lhsT, rhs, start, stop):
                nc.tensor.matmul(out, lhsT=lhsT, rhs=rhs, start=start, stop=stop)

            bias_t = persist.tile([128, 1], f32, tag="bias", name="bias_t")
            nc.vector.memset(bias_t[:], NEG_LOG_S)
            warm_t = persist.tile([128, 1], f32, tag="warm", name="warm_t")
            nc.scalar.activation(warm_t[:], bias_t[:], Sig, bias=bias_t[:])

            wq_sb = persist.tile([128, 1024], bf16, tag="wq", name="wq_sb")
            wk_sb = persist.tile([128, 1024], bf16, tag="wk", name="wk_sb")
            wv_sb = persist.tile([128, 1024], bf16, tag="wv", name="wv_sb")
            wo_sb = persist.tile([128, 1024], bf16, tag="wo", name="wo_sb")

            class BufSet:
                pass

            def make_bufset(s):
                bs = BufSet()
                bs.xt = [persist.tile([128, 2048], bf16, tag=f"xt{s}_{c}",
                                      name=f"xt{s}_{c}") for c in range(4)]
                bs.qt = [persist.tile([128, 2048], bf16, tag=f"qt{s}_{m}",
                                      name=f"qt{s}_{m}") for m in range(2)]
                bs.kt = [persist.tile([128, 2048], bf16, tag=f"kt{s}_{m}",
                                      name=f"kt{s}_{m}") for m in range(2)]
                bs.v = [persist.tile([128, 256], bf16, tag=f"v{s}_{t}",
                                     name=f"v{s}_{t}") for t in range(16)]
                return bs

            bsets = [make_bufset(s) for s in range(NB)]
            # ot is intra-iteration only: written by attention, read by the
            # interleaved output-projection waves of the same iteration.
            ot = [persist.tile([128, 2048], bf16, tag=f"ot{m}", name=f"ot{m}")
                  for m in range(2)]

            def dma_in(bs):
                for c in range(4):
                    nc.sync.dma_start(out=bs.xt[c][:],
                                      in_=xt_d[:, 2048 * c:2048 * (c + 1)])
                nc.sync.dma_start(out=wq_sb[:], in_=wq_d[:])
                nc.sync.dma_start(out=wk_sb[:], in_=wk_d[:])
                nc.sync.dma_start(out=wv_sb[:], in_=wv_d[:])
                nc.sync.dma_start(out=wo_sb[:], in_=wo_d[:])

            def proj_closures(bs):
                """32 closures, each emitting one PSUM group of the Q/K/V
                projections for buffer set bs. Spread across the previous
                iteration's attention loop."""
                fns = []
                for mc in range(2):
                    for w_sb, dsts in ((wq_sb, bs.qt), (wk_sb, bs.kt)):
                        for qc in range(4):
                            def f(mc=mc, w_sb=w_sb, dst=dsts[mc], qc=qc):
                                ps = spsum.tile([128, 512], f32, tag=ptag,
                                                name="pp", bufs=pbufs)
                                for kc in range(4):
                                    w0 = 256 * kc + 128 * mc
                                    mm(ps[:, :],
                                       w_sb[:, w0:w0 + 128],
                                       bs.xt[kc][:, 512 * qc:512 * (qc + 1)],
                                       start=(kc == 0), stop=(kc == 3))
                                nc.vector.tensor_copy(
                                    dst[:, 512 * qc:512 * (qc + 1)], ps[:, :])
                            fns.append(f)
                for tck in range(16):
                    def f(tck=tck):
                        pv = spsum.tile([128, 512], f32, tag=ptag,
                                        name="pv", bufs=pbufs)
                        for vkc in range(4):
                            mm(pv[:, 0:256],
                               bs.xt[vkc][:, 128 * tck:128 * (tck + 1)],
                               wv_sb[:, 256 * vkc:256 * (vkc + 1)],
                               start=(vkc == 0), stop=(vkc == 3))
                        nc.vector.tensor_copy(bs.v[tck][:], pv[:, 0:256])
                    fns.append(f)
                return fns

            def p_wave(wave):
                # output projection for tokens [256*wave, 256*(wave+1))
                st = stage.tile([128, 2, 512], f32, tag="pstage", name="pstage")
                for half in range(2):
                    ps = spsum.tile([128, 512], f32, tag=ptag,
                                    name="po", bufs=pbufs)
                    tck = 2 * wave + half
                    for c in range(2):
                        mm(ps[:, :],
                           ot[c][:, 128 * tck:128 * (tck + 1)],
                           wo_sb[:, 512 * c:512 * (c + 1)],
                           start=(c == 0), stop=(c == 1))
                    nc.vector.tensor_copy(st[:, half, :], ps[:, :])
                dst = out_d[256 * wave:256 * (wave + 1), :].rearrange(
                    "(t p) m -> p t m", p=128)
                nc.sync.dma_start(out=dst, in_=st[:])

            def attention(bs, inter):
                """Flat software-pipelined loop over groups of W score-matmul
                outputs; scores for group g+1 are emitted before sigmoid(g)/
                attnV(g) so the PE keeps ScalarE fed. The two heads of a pair
                run concurrently in the PE array (row-tiled scores at
                (0,0)/(64,0), col-tiled attnV at (0,0)/(0,64)).
                Output-projection waves run per finished qc. `inter` closures
                (next iteration's DMA + projections) are spread across the
                loop."""
                outs = [(qc, p, kc, h)
                        for qc in range(4) for p in range(2)
                        for kc in range(16) for h in range(2)]
                groups = [outs[i:i + W] for i in range(0, len(outs), W)]
                spread = {}
                if inter:
                    idxs = np.linspace(4, len(groups) - 6, len(inter)).astype(int)
                    for j, fn in enumerate(inter):
                        spread.setdefault(int(idxs[j]), []).append(fn)

                def emit_scores(grp):
                    sp = spsum.tile([128, 512 * len(grp)], f32, tag="s",
                                    name="ps")
                    for j, (qc, p, kc, h) in enumerate(grp):
                        mm(sp[:, 512 * j:512 * (j + 1)],
                           bs.kt[p][64 * h:64 * (h + 1), 128 * kc:128 * (kc + 1)],
                           bs.qt[p][64 * h:64 * (h + 1), 512 * qc:512 * (qc + 1)],
                           start=True, stop=True)
                    return sp

                psO = None
                s_cur = emit_scores(groups[0])
                for gi, grp in enumerate(groups):
                    s_nxt = emit_scores(groups[gi + 1]) \
                        if gi + 1 < len(groups) else None
                    a = apool.tile([128, 512 * len(grp)], bf16, tag="a",
                                   name="attn")
                    nc.scalar.activation(a[:], s_cur[:], Sig,
                                         bias=bias_t[:], scale=INV_SQRT_DK)
                    for j, (qc, p, kc, h) in enumerate(grp):
                        if kc == 0 and h == 0:
                            psO = opsum.tile([128, 512], f32, tag="o",
                                             name="psO")
                        pb = 128 * p + 64 * h
                        mm(psO[64 * h:64 * (h + 1), :],
                           bs.v[kc][:, pb:pb + 64],
                           a[:, 512 * j:512 * (j + 1)],
                           start=(kc == 0), stop=(kc == 15))
                        if kc == 15 and h == 1:
                            qs = slice(512 * qc, 512 * (qc + 1))
                            nc.vector.tensor_copy(ot[p][:, qs], psO[:, :])
                            if p == 1:
                                p_wave(2 * qc)
                                p_wave(2 * qc + 1)
                    for fn in spread.get(gi, []):
                        fn()
                    s_cur = s_nxt

            def body():
                # prologue: iteration 0's inputs + projections
                dma_in(bsets[0])
                for fn in proj_closures(bsets[0]):
                    fn()
                for k in range(iters):
                    cur = bsets[k % NB]
                    inter = []
                    if k + 1 < iters:
                        nxt = bsets[(k + 1) % NB]
                        inter = [lambda bs=nxt: dma_in(bs)] + proj_closures(nxt)
                    attention(cur, inter)

            if fori:
                hint = tuple(mybir.EngineType[e] for e in
                             ("PE", "Activation", "DVE", "SP", "Pool"))
                with tc.For_i(0, LOOP, 1, hint_engines=hint):
                    body()
            else:
                body()

    nc.compile()
    return nc


def get_nc():
    if "nc" not in _CACHE:
        _CACHE["nc"] = _build_nc()
    return _CACHE["nc"]


def make_in_maps(x, W_q, W_k, W_v, W_o):
    x = np.ascontiguousarray(np.asarray(x, dtype=np.float32))
    ws = [np.asarray(w, dtype=np.float32) for w in (W_q, W_k, W_v, W_o)]
    W_q, W_k, W_v, W_o = ws

    def chunked(a, nchunks):
        # [128*nchunks, m] -> [128, nchunks*m] with chunk-major columns
        m = a.shape[1]
        return np.ascontiguousarray(
            a.reshape(nchunks, 128, m).transpose(1, 0, 2).reshape(128, nchunks * m)
        ).astype(BF16_NP)

    in_maps = []
    for c in range(8):
        b, g = divmod(c, 2)
        gf = slice(GF * g, GF * (g + 1))
        in_maps.append({
            "xt": chunked(np.ascontiguousarray(x[b].T), 4),
            "wq": chunked(np.ascontiguousarray(W_q[gf, :].T), 4),
            "wk": chunked(np.ascontiguousarray(W_k[gf, :].T), 4),
            "wv": chunked(np.ascontiguousarray(W_v[gf, :].T), 4),
            "wo": chunked(np.ascontiguousarray(W_o[:, gf].T), 2),
        })
    return in_maps


def kernel(x, W_q, W_k, W_v, W_o):
    from concourse.bass_utils import run_bass_kernel_spmd

    nc = get_nc()
    in_maps = make_in_maps(x, W_q, W_k, W_v, W_o)
    res = run_bass_kernel_spmd(nc, in_maps, list(range(8)))
    parts = [res.results[c]["out"] for c in range(8)]
    out = np.stack([parts[2 * b] + parts[2 * b + 1] for b in range(B)])
    return np.ascontiguousarray(out.astype(np.float32))


# revision 17
# speedup vs baseline: 37.9294x; 1.0191x over previous
"""Sigmoid-attention MHA kernel for 8 Trainium2 NeuronCores.

Problem: x[4,2048,512], W_q/W_k/W_v/W_o[512,512] (already scaled).
  Q = x@Wq.T, K = x@Wk.T, V = x@Wv.T split into 8 heads of depth 64
  attn = sigmoid(QK^T/sqrt(64) - log(2048));  out = (attn@V merged)@Wo.T

Sharding: core c handles batch b=c//2, head-group g=c%2 (4 heads each).
Each core computes a partial output projection over its 256 head-features;
host sums the two partials per batch.

All PE operands are bf16 (host-converted); PSUM accumulation is fp32.
Attention matmuls use PE array tiling for 2x concurrency:
  scores: two heads' K=64 matmuls at row positions (0,0)/(64,0)
  attn@V: two heads' M=64 matmuls at col positions (0,0)/(0,64)
   (tile_position auto-derives from lhsT/out base partitions)
Sigmoid runs on ScalarE from PSUM [128,1024] tiles -> bf16 SBUF; ScalarE
is the bottleneck engine (~1.1us per tile x 128 tiles per iteration).

KERNEL_LOOP>0 unrolls the body N times (python loop, no barriers) with
cross-iteration software pipelining: iteration k+1's input DMA and
Q/K/V projections are interleaved into iteration k's attention loop
(x/q/k/v tiles double-buffered), so a timing run measures steady-state
throughput.
"""

import os
import numpy as np
import ml_dtypes

LOOP = int(os.environ.get("KERNEL_LOOP", "0"))  # For_i trip count (timing)
UNROLL = int(os.environ.get("KERNEL_UNROLL", "16"))  # bodies per For_i trip
ABUFS = int(os.environ.get("KERNEL_AB", "6"))   # attn sbuf bufs
# sigmoid tile width (512-col score matmul outputs per ScalarE activation);
# 1536 amortizes the ~350-cycle per-instruction ACT overhead over 3 outputs.
ACTW = int(os.environ.get("KERNEL_ACTW", "1024"))
W = ACTW // 512
# Per (qc,p) block of 16 score tiles, stage the first 4*NSTG tiles
# PSUM->SBUF on the DVE so ScalarE runs one 4096-wide sigmoid per 4 tiles
# (amortizing its per-instruction overhead) instead of 4 PSUM-sourced ones.
NSTG = int(os.environ.get("KERNEL_NSTG", "0"))

B, S, D = 4, 2048, 512
NH, DEPTH = 8, 64
G = 2          # head groups (one per core pair)
GF = 256       # features per group
NEG_LOG_S = float(np.float32(-np.log(np.float32(S))))
INV_SQRT_DK = 0.125
BF16_NP = np.dtype(ml_dtypes.bfloat16)

_CACHE = {}


def _build_nc():
    import concourse.bacc as bacc
    import concourse.tile as tile
    from concourse import mybir

    f32 = mybir.dt.float32
    bf16 = mybir.dt.bfloat16
    nc = bacc.Bacc("TRN2", target_bir_lowering=False, debug=False, num_devices=8)

    xt_d = nc.dram_tensor("xt", [128, 8192], bf16, kind="ExternalInput").ap()
    wq_d = nc.dram_tensor("wq", [128, 1024], bf16, kind="ExternalInput").ap()
    wk_d = nc.dram_tensor("wk", [128, 1024], bf16, kind="ExternalInput").ap()
    wv_d = nc.dram_tensor("wv", [128, 1024], bf16, kind="ExternalInput").ap()
    wo_d = nc.dram_tensor("wo", [128, 1024], bf16, kind="ExternalInput").ap()
    out_d = nc.dram_tensor("out", [S, D], f32, kind="ExternalOutput").ap()

    # LOOP>0: For_i(0, LOOP) around UNROLL pipelined bodies, so a timing run
    # measures steady-state throughput with the loop barrier amortized 1/UNROLL.
    # KERNEL_FORI=0: python-unroll LOOP bodies instead (TimelineSim can't
    # follow For_i register branches).
    fori = bool(int(os.environ.get("KERNEL_FORI", "1"))) and LOOP > 0
    iters = UNROLL if fori else max(LOOP, 1)
    NB = 2 if iters > 1 else 1  # buffer sets for cross-iteration pipelining

    # PSUM budget (8 banks of 512 fp32):
    #   W=3: scores [128,1536] x2 (6 banks) + psO [128,512] x1 + proj x1
    #   W=2: scores [128,1024] x3 (6 banks) + psO [128,512] x2, proj shares "s"
    sb = {2: 3, 3: 2}[W]
    ob = {2: 2, 3: 1}[W]
    ptag = "s" if W == 2 else "pj"
    pbufs = None if W == 2 else 1  # the pj tag gets a single PSUM bank

    with tile.TileContext(nc) as tc:
        with (
            tc.tile_pool(name="persist", bufs=1) as persist,
            tc.tile_pool(name="attn", bufs=ABUFS) as apool,
            tc.tile_pool(name="stage", bufs=4) as stage,
            tc.tile_pool(name="spsum", bufs=sb, space="PSUM") as spsum,
            tc.tile_pool(name="opsum", bufs=ob, space="PSUM") as opsum,
        ):
            Sig = mybir.ActivationFunctionType.Sigmoid

            def mm(out, lhsT, rhs, start, stop):
                nc.tensor.matmul(out, lhsT=lhsT, rhs=rhs, start=start, stop=stop)

            bias_t = persist.tile([128, 1], f32, tag="bias", name="bias_t")
            nc.vector.memset(bias_t[:], NEG_LOG_S)
            warm_t = persist.tile([128, 1], f32, tag="warm", name="warm_t")
            nc.scalar.activation(warm_t[:], bias_t[:], Sig, bias=bias_t[:])

            wq_sb = persist.tile([128, 1024], bf16, tag="wq", name="wq_sb")
            wk_sb = persist.tile([128, 1024], bf16, tag="wk", name="wk_sb")
            wv_sb = persist.tile([128, 1024], bf16, tag="wv", name="wv_sb")
            wo_sb = persist.tile([128, 1024], bf16, tag="wo", name="wo_sb")

            class BufSet:
                pass

            def make_bufset(s):
                bs = BufSet()
                bs.xt = [persist.tile([128, 2048], bf16, tag=f"xt{s}_{c}",
                                      name=f"xt{s}_{c}") for c in range(4)]
                bs.qt = [persist.tile([128, 2048], bf16, tag=f"qt{s}_{m}",
                                      name=f"qt{s}_{m}") for m in range(2)]
                bs.kt = [persist.tile([128, 2048], bf16, tag=f"kt{s}_{m}",
                                      name=f"kt{s}_{m}") for m in range(2)]
                bs.v = [persist.tile([128, 256], bf16, tag=f"v{s}_{t}",
                                     name=f"v{s}_{t}") for t in range(16)]
                return bs

            bsets = [make_bufset(s) for s in range(NB)]
            # ot is intra-iteration only: written by attention, read by the
            # interleaved output-projection waves of the same iteration.
            ot = [persist.tile([128, 2048], bf16, tag=f"ot{m}", name=f"ot{m}")
                  for m in range(2)]

            def dma_in(bs):
                for c in range(4):
                    nc.sync.dma_start(out=bs.xt[c][:],
                                      in_=xt_d[:, 2048 * c:2048 * (c + 1)])
                nc.sync.dma_start(out=wq_sb[:], in_=wq_d[:])
                nc.sync.dma_start(out=wk_sb[:], in_=wk_d[:])
                nc.sync.dma_start(out=wv_sb[:], in_=wv_d[:])
                nc.sync.dma_start(out=wo_sb[:], in_=wo_d[:])

            def proj_closures(bs):
                """32 closures, each emitting one PSUM group of the Q/K/V
                projections for buffer set bs. Spread across the previous
                iteration's attention loop."""
                fns = []
                for mc in range(2):
                    for w_sb, dsts in ((wq_sb, bs.qt), (wk_sb, bs.kt)):
                        for qc in range(4):
                            def f(mc=mc, w_sb=w_sb, dst=dsts[mc], qc=qc):
                                ps = spsum.tile([128, 512], f32, tag=ptag,
                                                name="pp", bufs=pbufs)
                                for kc in range(4):
                                    w0 = 256 * kc + 128 * mc
                                    mm(ps[:, :],
                                       w_sb[:, w0:w0 + 128],
                                       bs.xt[kc][:, 512 * qc:512 * (qc + 1)],
                                       start=(kc == 0), stop=(kc == 3))
                                nc.vector.tensor_copy(
                                    dst[:, 512 * qc:512 * (qc + 1)], ps[:, :])
                            fns.append(f)
                for tck in range(16):
                    def f(tck=tck):
                        pv = spsum.tile([128, 512], f32, tag=ptag,
                                        name="pv", bufs=pbufs)
                        for vkc in range(4):
                            mm(pv[:, 0:256],
                               bs.xt[vkc][:, 128 * tck:128 * (tck + 1)],
                               wv_sb[:, 256 * vkc:256 * (vkc + 1)],
                               start=(vkc == 0), stop=(vkc == 3))
                        nc.vector.tensor_copy(bs.v[tck][:], pv[:, 0:256])
                    fns.append(f)
                return fns

            def p_wave(wave):
                # output projection for tokens [256*wave, 256*(wave+1))
                st = stage.tile([128, 2, 512], f32, tag="pstage", name="pstage")
                for half in range(2):
                    ps = spsum.tile([128, 512], f32, tag=ptag,
                                    name="po", bufs=pbufs)
                    tck = 2 * wave + half
                    for c in range(2):
                        mm(ps[:, :],
                           ot[c][:, 128 * tck:128 * (tck + 1)],
                           wo_sb[:, 512 * c:512 * (c + 1)],
                           start=(c == 0), stop=(c == 1))
                    nc.vector.tensor_copy(st[:, half, :], ps[:, :])
                dst = out_d[256 * wave:256 * (wave + 1), :].rearrange(
                    "(t p) m -> p t m", p=128)
                nc.sync.dma_start(out=dst, in_=st[:])

            def attention(bs, inter):
                """Flat software-pipelined loop over groups of W score-matmul
                outputs; scores for group g+1 are emitted before sigmoid(g)/
                attnV(g) so the PE keeps ScalarE fed. The two heads of a pair
                run concurrently in the PE array (row-tiled scores at
                (0,0)/(64,0), col-tiled attnV at (0,0)/(0,64)).
                Output-projection waves run per finished qc. `inter` closures
                (next iteration's DMA + projections) are spread across the
                loop."""
                outs = [(qc, p, kc, h)
                        for qc in range(4) for p in range(2)
                        for kc in range(16) for h in range(2)]
                groups = [outs[i:i + W] for i in range(0, len(outs), W)]
                spread = {}
                if inter:
                    idxs = np.linspace(4, len(groups) - 6, len(inter)).astype(int)
                    for j, fn in enumerate(inter):
                        spread.setdefault(int(idxs[j]), []).append(fn)

                def emit_scores(grp):
                    sp = spsum.tile([128, 512 * len(grp)], f32, tag="s",
                                    name="ps")
                    for j, (qc, p, kc, h) in enumerate(grp):
                        mm(sp[:, 512 * j:512 * (j + 1)],
                           bs.kt[p][64 * h:64 * (h + 1), 128 * kc:128 * (kc + 1)],
                           bs.qt[p][64 * h:64 * (h + 1), 512 * qc:512 * (qc + 1)],
                           start=True, stop=True)
                    return sp

                psO = None
                stg = None

                def attn_v(a_ap, qc, p, kc, h):
                    nonlocal psO
                    if kc == 0 and h == 0:
                        psO = opsum.tile([128, 512], f32, tag="o", name="psO")
                    pb = 128 * p + 64 * h
                    mm(psO[64 * h:64 * (h + 1), :],
                       bs.v[kc][:, pb:pb + 64], a_ap,
                       start=(kc == 0), stop=(kc == 15))
                    if kc == 15 and h == 1:
                        qs = slice(512 * qc, 512 * (qc + 1))
                        nc.vector.tensor_copy(ot[p][:, qs], psO[:, :])
                        if p == 1:
                            p_wave(2 * qc)
                            p_wave(2 * qc + 1)

                s_cur = emit_scores(groups[0])
                for gi, grp in enumerate(groups):
                    s_nxt = emit_scores(groups[gi + 1]) \
                        if gi + 1 < len(groups) else None
                    kc0 = grp[0][2]
                    if W == 2 and kc0 < 4 * NSTG:
                        # staged: DVE copies the PSUM tile out; one 4096-wide
                        # SBUF-sourced sigmoid per 4 tiles
                        j4 = kc0 % 4
                        if j4 == 0:
                            stg = stage.tile([128, 4096], f32, tag="stg",
                                             name="stg", bufs=2)
                        nc.vector.tensor_copy(
                            stg[:, 1024 * j4:1024 * (j4 + 1)], s_cur[:, :])
                        if j4 == 3:
                            a4 = apool.tile([128, 4096], bf16, tag="a4",
                                            name="attn4", bufs=2)
                            nc.scalar.activation(a4[:], stg[:], Sig,
                                                 bias=bias_t[:],
                                                 scale=INV_SQRT_DK)
                            qc, p = grp[0][0], grp[0][1]
                            for jj in range(4):
                                for h in range(2):
                                    attn_v(a4[:, 1024 * jj + 512 * h:
                                              1024 * jj + 512 * (h + 1)],
                                           qc, p, kc0 - 3 + jj, h)
                    else:
                        a = apool.tile([128, 512 * len(grp)], bf16, tag="a",
                                       name="attn")
                        nc.scalar.activation(a[:], s_cur[:], Sig,
                                             bias=bias_t[:], scale=INV_SQRT_DK)
                        for j, (qc, p, kc, h) in enumerate(grp):
                            attn_v(a[:, 512 * j:512 * (j + 1)], qc, p, kc, h)
                    for fn in spread.get(gi, []):
                        fn()
                    s_cur = s_nxt

            def body():
                # prologue: iteration 0's inputs + projections
                dma_in(bsets[0])
                for fn in proj_closures(bsets[0]):
                    fn()
                for k in range(iters):
                    cur = bsets[k % NB]
                    inter = []
                    if k + 1 < iters:
                        nxt = bsets[(k + 1) % NB]
                        inter = [lambda bs=nxt: dma_in(bs)] + proj_closures(nxt)
                    attention(cur, inter)

            if fori:
                hint = tuple(mybir.EngineType[e] for e in
                             ("PE", "Activation", "DVE", "SP", "Pool"))
                stag = bool(int(os.environ.get("KERNEL_STAG", "0")))
                with tc.For_i(0, LOOP, 1, hint_engines=hint,
                              staggered_reset=stag):
                    body()
            else:
                body()

    nc.compile()
    return nc


def get_nc():
    if "nc" not in _CACHE:
        _CACHE["nc"] = _build_nc()
    return _CACHE["nc"]


def make_in_maps(x, W_q, W_k, W_v, W_o):
    x = np.ascontiguousarray(np.asarray(x, dtype=np.float32))
    ws = [np.asarray(w, dtype=np.float32) for w in (W_q, W_k, W_v, W_o)]
    W_q, W_k, W_v, W_o = ws

    def chunked(a, nchunks):
        # [128*nchunks, m] -> [128, nchunks*m] with chunk-major columns
        m = a.shape[1]
        return np.ascontiguousarray(
            a.reshape(nchunks, 128, m).transpose(1, 0, 2).reshape(128, nchunks * m)
        ).astype(BF16_NP)

    in_maps = []
    for c in range(8):
        b, g = divmod(c, 2)
        gf = slice(GF * g, GF * (g + 1))
        in_maps.append({
            "xt": chunked(np.ascontiguousarray(x[b].T), 4),
            "wq": chunked(np.ascontiguousarray(W_q[gf, :].T), 4),
            "wk": chunked(np.ascontiguousarray(W_k[gf, :].T), 4),
            "wv": chunked(np.ascontiguousarray(W_v[gf, :].T), 4),
            "wo": chunked(np.ascontiguousarray(W_o[:, gf].T), 2),
        })
    return in_maps


def kernel(x, W_q, W_k, W_v, W_o):
    from concourse.bass_utils import run_bass_kernel_spmd

    nc = get_nc()
    in_maps = make_in_maps(x, W_q, W_k, W_v, W_o)
    res = run_bass_kernel_spmd(nc, in_maps, list(range(8)))
    parts = [res.results[c]["out"] for c in range(8)]
    out = np.stack([parts[2 * b] + parts[2 * b + 1] for b in range(B)])
    return np.ascontiguousarray(out.astype(np.float32))
